# revision 1
# baseline (speedup 1.0000x reference)
"""Trainium2 Bass kernel for nn_DiffusionModel (GCN diffusion model).

Strategy (8 NeuronCores, SPMD one NEFF):
- Nodes sharded contiguously: core c owns nodes [c*6250, (c+1)*6250).
- h kept feature-major in SBUF: h_fm[128, k, node] = h[node, 128k+p].
- Per GNN layer:
    xw = h @ gcn_w computed node-major (lhsT = h_fm slice), scaled by
    dinv[node], cast fp16 -> AllGather into a full [50000, 256] fp16 table
    in local DRAM (table rows are dinv[s]*xw[s]).
    Destination-sorted edges (incl. self loops), grouped by dst-block of 128
    and by src-half (int16 gather index limit): dma_gather fetches the
    src rows; one-hot fp16 S-matrices (host-built, streamed from DRAM)
    reduce each 128-edge chunk into PSUM [dst, 256] via TensorE matmul.
    agg *= dinv[dst] (per-partition scalar), transpose 2x[128,128] on PE,
    accumulate into h_fm; gcn_b added once per layer.
    Residual MLP blocks run feature-major; LayerNorm over the feature
    (partition) axis uses ones-matmul reductions + PE row broadcasts.
- Output projection writes y feature-major [128, 6250]; host transposes.

All edge bookkeeping (degrees, dinv, sort, padding, one-hot S, int16 index
wrapping) is host-side numpy preprocessing of the integer edge list.
"""
import sys

for _p in ("/opt/trn_rl_repo",):
    if _p not in sys.path:
        sys.path.insert(0, _p)

import numpy as np

N = 50000
E = 800000
D = 128
H = 256
L = 4
C = 8            # cores
S = N // C       # 6250 nodes per core
NBLK = (S + 127) // 128   # 49 dst blocks per core
SUB0 = 3200      # local-node split (25 blocks); sub-tables fit int16
NCHUNK = 512     # node chunk for dense/MLP phases
EPS = 1e-5

_FP16 = np.float16


def _silu(x):
    return x / (1.0 + np.exp(-x))


def _round_up(x, m):
    return (x + m - 1) // m * m


def preprocess(edge_index):
    """Host-side integer preprocessing. Returns (shared, per_core) where
    shared holds the compile-time-constant paddings/offsets and per_core
    holds each core's idx / S / dinv arrays."""
    loops = np.arange(N, dtype=np.int64)
    src = np.concatenate([np.asarray(edge_index[0], np.int64), loops])
    dst = np.concatenate([np.asarray(edge_index[1], np.int64), loops])
    deg = np.bincount(dst, minlength=N).astype(np.float64)  # incl. self loop
    dinv = (1.0 / np.sqrt(deg)).astype(np.float32)

    # group edges per (core, block, half) with self loops appended
    groups = [[None] * 2 for _ in range(C * NBLK)]  # flat [core*NBLK+b][h]
    order = np.argsort(dst, kind="stable")
    ssrc, sdst = src[order], dst[order]
    core_bounds = np.searchsorted(sdst, np.arange(C + 1) * S)
    for c in range(C):
        lo, hi = core_bounds[c], core_bounds[c + 1]
        es = ssrc[lo:hi].copy()
        ed = sdst[lo:hi] - c * S
        blk = ed >> 7
        half = ((es % S) >= SUB0).astype(np.int64)
        dloc = ed & 127
        key = blk * 2 + half
        o2 = np.argsort(key, kind="stable")
        es, dloc, key = es[o2], dloc[o2], key[o2]
        bounds = np.searchsorted(key, np.arange(2 * NBLK + 1))
        for b in range(NBLK):
            for h in range(2):
                g0, g1 = bounds[b * 2 + h], bounds[b * 2 + h + 1]
                e = es[g0:g1]
                if h == 0:
                    sec_i = (e // S) * SUB0 + (e % S)
                else:
                    sec_i = (e // S) * (S - SUB0) + (e % S) - SUB0
                sec_d = dloc[g0:g1]
                o3 = np.argsort(sec_i, kind="stable")
                groups[c * NBLK + b][h] = (sec_i[o3], sec_d[o3])

    # shared paddings: max count over cores, rounded to 128
    NBH = np.zeros((NBLK, 2), np.int64)
    for b in range(NBLK):
        for h in range(2):
            mx = max(len(groups[c * NBLK + b][h][0]) for c in range(C))
            NBH[b, h] = _round_up(mx, 128) if mx > 0 else 0

    # group blocks so gather calls are large (amortize SWDGE fixed cost)
    GRP = 4
    grp_blocks = [list(range(g, min(g + GRP, NBLK)))
                  for g in range(0, NBLK, GRP)]
    ngrp = len(grp_blocks)
    na = np.zeros(ngrp, np.int64)     # idx per group, half A
    nb_ = np.zeros(ngrp, np.int64)    # idx per group, half B
    a_pos = np.zeros(NBLK, np.int64)  # chunk offset of block's A sect in group
    b_pos = np.zeros(NBLK, np.int64)
    for g, bl in enumerate(grp_blocks):
        pos = 0
        for b in bl:
            a_pos[b] = pos
            pos += NBH[b, 0] // 128
        na[g] = pos * 128
        for b in bl:
            b_pos[b] = pos
            pos += NBH[b, 1] // 128
        nb_[g] = pos * 128 - na[g]
    gchunks = (na + nb_) // 128
    gbase = np.zeros(ngrp, np.int64)
    idx_off = np.zeros((ngrp, 2), np.int64)
    acc_c = 0
    acc_i = 0
    for g in range(ngrp):
        gbase[g] = acc_c
        acc_c += gchunks[g]
        idx_off[g, 0] = acc_i
        acc_i += na[g] // 16
        idx_off[g, 1] = acc_i
        acc_i += nb_[g] // 16
    icols = acc_i
    nch_tot = acc_c

    shared = dict(NBH=NBH, icols=int(icols), nch_tot=int(nch_tot),
                  grp_blocks=grp_blocks, na=na, nb=nb_, a_pos=a_pos,
                  b_pos=b_pos, gchunks=gchunks, gbase=gbase,
                  idx_off=idx_off, gch_max=int(gchunks.max()),
                  ch_max=int(((NBH[:, 0] + NBH[:, 1]) // 128).max()))

    per_core = []
    for c in range(C):
        idx_arr = np.zeros((16, icols), np.int16)
        dloc_arr = np.full((128, nch_tot), 255.0, _FP16)
        for g, bl in enumerate(grp_blocks):
            for h in range(2):
                vals = []
                for b in bl:
                    g_idx, g_dloc = groups[c * NBLK + b][h]
                    n = len(g_idx)
                    nbh = NBH[b, h]
                    v = np.zeros(nbh, np.int16)
                    v[:n] = g_idx.astype(np.int16)   # pads gather row 0
                    vals.append(v)
                    # dloc: chunk position of this section within the group
                    sect = a_pos[b] if h == 0 else b_pos[b]
                    jj = np.arange(n)
                    dloc_arr[jj % 128, gbase[g] + sect + jj // 128] = \
                        g_dloc.astype(_FP16)
                if not vals:
                    continue
                vv = np.concatenate(vals)
                nt = len(vv)
                if nt == 0:
                    continue
                j = np.arange(nt)
                ia = np.zeros((16, nt // 16), np.int16)
                ia[j % 16, j // 16] = vv
                o = idx_off[g, h]
                idx_arr[:, o:o + nt // 16] = ia
        flat = np.zeros(128 * NBLK, np.float32)
        flat[:S] = dinv[c * S:(c + 1) * S]
        dv = np.ascontiguousarray(flat.reshape(NBLK, 128).T)  # dv[p, b]
        per_core.append(dict(
            idx=np.tile(idx_arr, (8, 1)),
            dloc=dloc_arr,
            dinv=dv,
        ))
    return shared, per_core


def build_colvecs(inputs):
    """Per-feature vectors packed as [128, ncols] fp32 + index map."""
    t = np.asarray(inputs["t"], np.float32)
    te = _silu(t[:, None] @ inputs["time_w1"] + inputs["time_b1"]) \
        @ inputs["time_w2"] + inputs["time_b2"]
    bias_in = (np.asarray(inputs["in_b"], np.float32) + te[0]).astype(np.float32)

    cols = []
    cmap = {}

    def add(name, vec):
        vec = np.asarray(vec, np.float32).reshape(-1)
        k = len(vec) // 128
        cmap[name] = len(cols)
        for i in range(k):
            cols.append(vec[i * 128:(i + 1) * 128])

    add("bias_in", bias_in)
    for l in range(L):
        add(f"gcn_b{l}", inputs["gcn_b"][l])
        add(f"res_b1{l}", inputs["res_b1"][l])
        add(f"res_g1{l}", inputs["res_g1"][l])
        add(f"res_be1{l}", inputs["res_be1"][l])
        add(f"res_b2{l}", inputs["res_b2"][l])
        add(f"res_g2{l}", inputs["res_g2"][l])
        add(f"res_be2{l}", inputs["res_be2"][l])
    add("out_b1", inputs["out_b1"])
    add("out_b2", inputs["out_b2"])
    return np.stack(cols, axis=1), cmap


def build_module(shared, cmap, rmap=None, reps=1, single_core=False,
                 skip_agg=False, skip_mlp=False, skip_gather=False,
                 agg_dma_only=False,
                 local_table=False, st_unmerged=False, tp32=False):
    import concourse.bacc as bacc
    import concourse.mybir as mybir
    import concourse.tile as tile
    from concourse.masks import make_identity

    dt = mybir.dt
    Alu = mybir.AluOpType
    Act = mybir.ActivationFunctionType

    NBH = shared["NBH"]
    idx_off = shared["idx_off"]
    icols, ch_max = shared["icols"], shared["ch_max"]
    ncv = cmap["ncols"]

    nc = bacc.Bacc("TRN2", target_bir_lowering=False, debug=False,
                   num_devices=1 if single_core else C,
                   num_swdge_queues=4)
    # ---- I/O ----
    xT_in = nc.dram_tensor("xT", [D, S], dt.float16, kind="ExternalInput")
    idx_in = nc.dram_tensor("idx", [128, icols], dt.int16, kind="ExternalInput")
    dloc_in = nc.dram_tensor("dloc", [128, shared["nch_tot"]], dt.float16,
                             kind="ExternalInput")
    dinv_in = nc.dram_tensor("dinv", [128, NBLK], dt.float32, kind="ExternalInput")
    iota_in = nc.dram_tensor("iota", [128, 128], dt.float16, kind="ExternalInput")
    cv_in = nc.dram_tensor("cv", [128, ncv], dt.float32, kind="ExternalInput")
    in_w_in = nc.dram_tensor("in_w", [D, H], dt.float16, kind="ExternalInput")
    gcn_w_in = nc.dram_tensor("gcn_w", [L, H, H], dt.float16, kind="ExternalInput")
    res_w1_in = nc.dram_tensor("res_w1", [L, H, H], dt.float16, kind="ExternalInput")
    res_w2_in = nc.dram_tensor("res_w2", [L, H, H], dt.float16, kind="ExternalInput")
    out_w1_in = nc.dram_tensor("out_w1", [H, H], dt.float16, kind="ExternalInput")
    out_w2_in = nc.dram_tensor("out_w2", [H, D], dt.float16, kind="ExternalInput")
    y_out = nc.dram_tensor("y", [D, S], dt.float32, kind="ExternalOutput")

    chunks = [(i * NCHUNK, min(NCHUNK, S - i * NCHUNK))
              for i in range((S + NCHUNK - 1) // NCHUNK)]
    blocks = [(b, b * 128, min(128, S - b * 128)) for b in range(NBLK)]

    with tile.TileContext(nc) as tc:
        with tc.tile_pool(name="const", bufs=1) as constp, \
             tc.tile_pool(name="hpool", bufs=1) as hpool, \
             tc.tile_pool(name="wpool", bufs=1) as wpool, \
             tc.tile_pool(name="dram", bufs=1, space="DRAM") as dram:

            ident = constp.tile([128, 128],
                                dt.float32 if tp32 else dt.float16)
            make_identity(nc, ident[:])
            ones_col = constp.tile([128, 1], dt.float16)
            nc.vector.memset(ones_col[:], 1.0)
            ones_row = constp.tile([1, 128], dt.float16)
            nc.vector.memset(ones_row[:], 1.0)
            eps_sc = constp.tile([1, 1], dt.float32)
            nc.vector.memset(eps_sc[:], EPS)
            cv = constp.tile([128, ncv], dt.float32)
            nc.sync.dma_start(out=cv[:], in_=cv_in[:, :])
            dinv_sb = constp.tile([128, NBLK], dt.float32)
            nc.sync.dma_start(out=dinv_sb[:], in_=dinv_in[:, :])
            iota_sb = constp.tile([128, 128], dt.float16)
            nc.sync.dma_start(out=iota_sb[:], in_=iota_in[:, :])
            dloc_sb = constp.tile([128, shared["nch_tot"]], dt.float16)
            nc.sync.dma_start(out=dloc_sb[:], in_=dloc_in[:, :])
            idx_sb = constp.tile([128, icols], dt.int16)
            nc.sync.dma_start(out=idx_sb[:], in_=idx_in[:, :])
            in_w_sb = constp.tile([128, H], dt.float16)
            nc.sync.dma_start(out=in_w_sb[:], in_=in_w_in[:, :])
            out_w1_sb = constp.tile([128, 2, H], dt.float16)
            nc.sync.dma_start(out=out_w1_sb[:],
                              in_=out_w1_in[:, :].rearrange("(k p) h -> p k h", p=128))
            out_w2_sb = constp.tile([128, 2, D], dt.float16)
            nc.sync.dma_start(out=out_w2_sb[:],
                              in_=out_w2_in[:, :].rearrange("(k p) h -> p k h", p=128))

            h_fm = hpool.tile([128, 2, S], dt.float16)

            ag_in0 = dram.tile([SUB0, H], dt.float16)
            ag_in1 = dram.tile([S - SUB0, H], dt.float16)

            def col(name):
                return cv[:, cmap[name]:cmap[name] + 1]

            def colk(name, k):
                i = cmap[name] + k
                return cv[:, i:i + 1]

            for rep in range(reps):
                # ================= input projection =================
                with tc.tile_pool(name="ph0", bufs=3, space="PSUM") as psum0, \
                     tc.tile_pool(name="xtp", bufs=1) as xtp:
                    xT_sb = xtp.tile([128, S], dt.float16)
                    nc.sync.dma_start(out=xT_sb[:], in_=xT_in[:, :])
                    for n0, nn in chunks:
                        for m in range(2):
                            ps = psum0.tile([128, NCHUNK], dt.float32, tag="ps")
                            nc.tensor.matmul(
                                out=ps[:, :nn],
                                lhsT=in_w_sb[:, m * 128:(m + 1) * 128],
                                rhs=xT_sb[:, n0:n0 + nn],
                                start=True, stop=True)
                            nc.vector.tensor_scalar_add(
                                h_fm[:, m, n0:n0 + nn], ps[:, :nn],
                                colk("bias_in", m))

                # ================= GNN layers =================
                with tc.tile_pool(name="lw", bufs=1) as lw, \
                     tc.tile_pool(name="pa", bufs=2, space="PSUM") as pa, \
                     tc.tile_pool(name="pk", bufs=4, space="PSUM") as pk, \
                     tc.tile_pool(name="gtp", bufs=2) as gtp, \
                     tc.tile_pool(name="stp", bufs=3) as stp, \
                     tc.tile_pool(name="xwp", bufs=3) as xwp, \
                     tc.tile_pool(name="mt", bufs=2) as mt, \
                     tc.tile_pool(name="xwall", bufs=1) as xwall:
                    for l in range(L):
                        xws_all = xwall.tile([128, NBLK, H], dt.float16,
                                             tag="xws_all")
                        w_gcn = lw.tile([128, 2, H], dt.float16)
                        nc.sync.dma_start(
                            out=w_gcn[:],
                            in_=gcn_w_in[l].rearrange("(k p) h -> p k h", p=128))
                        w1 = lw.tile([128, 2, H], dt.float16)
                        nc.sync.dma_start(
                            out=w1[:],
                            in_=res_w1_in[l].rearrange("(k p) h -> p k h", p=128))
                        w2 = lw.tile([128, 2, H], dt.float16)
                        nc.sync.dma_start(
                            out=w2[:],
                            in_=res_w2_in[l].rearrange("(k p) h -> p k h", p=128))

                        # ---- A: table shard = dinv * (h @ gcn_w), fp16 ----
                        def partA_block(b, r0, rn):
                            ps = pk.tile([128, NCHUNK], dt.float32, tag="pk")
                            for k in range(2):
                                nc.tensor.matmul(
                                    out=ps[:rn, :H],
                                    lhsT=h_fm[:, k, r0:r0 + rn],
                                    rhs=w_gcn[:, k, :],
                                    start=(k == 0), stop=(k == 1))
                            nc.vector.tensor_scalar(
                                out=xws_all[:rn, b, :], in0=ps[:rn, :H],
                                scalar1=dinv_sb[:rn, b:b + 1], scalar2=None,
                                op0=Alu.mult)
                            if r0 < SUB0:
                                nc.sync.dma_start(out=ag_in0[r0:r0 + rn, :],
                                                  in_=xws_all[:rn, b, :])
                            else:
                                nc.sync.dma_start(
                                    out=ag_in1[r0 - SUB0:r0 - SUB0 + rn, :],
                                    in_=xws_all[:rn, b, :])

                        # ---- B: split AllGather, half 0 issued early ----
                        table0 = dram.tile([C * SUB0, H], dt.float16,
                                           addr_space="Local"
                                           if (single_core or local_table)
                                           else "Shared", tag="table0",
                                           name=f"table0_{rep}_{l}")
                        table1 = dram.tile([C * (S - SUB0), H], dt.float16,
                                           addr_space="Local"
                                           if (single_core or local_table)
                                           else "Shared", tag="table1",
                                           name=f"table1_{rep}_{l}")
                        nsub0 = SUB0 // 128
                        for b, r0, rn in blocks[:nsub0]:
                            partA_block(b, r0, rn)
                        if single_core or local_table:
                            nc.sync.dma_start(out=table0[0:SUB0, :],
                                              in_=ag_in0[:, :])
                        else:
                            nc.gpsimd.collective_compute(
                                "AllGather", Alu.bypass,
                                replica_groups=[list(range(C))],
                                ins=[ag_in0[:, :]],
                                outs=[table0[:, :]],
                            )
                        for b, r0, rn in blocks[nsub0:]:
                            partA_block(b, r0, rn)
                        if single_core or local_table:
                            nc.sync.dma_start(out=table1[0:S - SUB0, :],
                                              in_=ag_in1[:, :])
                        else:
                            nc.gpsimd.collective_compute(
                                "AllGather", Alu.bypass,
                                replica_groups=[list(range(C))],
                                ins=[ag_in1[:, :]],
                                outs=[table1[:, :]],
                            )

                        # ---- res-MLP for one node chunk (chunk-local) ----
                        def mlp_chunk(n0, nn):
                            u = mt.tile([128, 2, NCHUNK], dt.float16, tag="u")
                            for m in range(2):
                                ps = pk.tile([128, NCHUNK], dt.float32,
                                             tag="pk")
                                for k in range(2):
                                    nc.tensor.matmul(
                                        out=ps[:, :nn],
                                        lhsT=w1[:, k, m * 128:(m + 1) * 128],
                                        rhs=h_fm[:, k, n0:n0 + nn],
                                        start=(k == 0), stop=(k == 1))
                                nc.vector.tensor_scalar_add(
                                    u[:, m, :nn], ps[:, :nn],
                                    colk(f"res_b1{l}", m))
                            r1 = mt.tile([128, 2, NCHUNK], dt.float16,
                                         tag="r1")
                            layernorm(u, nn, f"res_g1{l}", f"res_be1{l}",
                                      r1, True)
                            u2 = mt.tile([128, 2, NCHUNK], dt.float16,
                                         tag="u")
                            for m in range(2):
                                ps = pk.tile([128, NCHUNK], dt.float32,
                                             tag="pk")
                                for k in range(2):
                                    nc.tensor.matmul(
                                        out=ps[:, :nn],
                                        lhsT=w2[:, k, m * 128:(m + 1) * 128],
                                        rhs=r1[:, k, :nn],
                                        start=(k == 0), stop=(k == 1))
                                nc.vector.tensor_scalar_add(
                                    u2[:, m, :nn], ps[:, :nn],
                                    colk(f"res_b2{l}", m))
                            r2 = mt.tile([128, 2, NCHUNK], dt.float16,
                                         tag="r1")
                            layernorm(u2, nn, f"res_g2{l}", f"res_be2{l}",
                                      r2, False)
                            for m in range(2):
                                nc.vector.tensor_tensor(
                                    out=h_fm[:, m, n0:n0 + nn],
                                    in0=h_fm[:, m, n0:n0 + nn],
                                    in1=r2[:, m, :nn], op=Alu.add)

                        def layernorm(u_sb, nn, gname, bename, out_sb, silu):
                            sq = mt.tile([128, 2, NCHUNK], dt.float16,
                                         tag="sq", bufs=2)
                            nc.scalar.activation(sq[:, :, :nn],
                                                 u_sb[:, :, :nn], Act.Square)
                            mean_ps = pk.tile([1, NCHUNK], dt.float32,
                                              tag="pk")
                            ssq_ps = pk.tile([1, NCHUNK], dt.float32,
                                             tag="pk")
                            for m in range(2):
                                nc.tensor.matmul(
                                    out=mean_ps[:, :nn], lhsT=ones_col[:],
                                    rhs=u_sb[:, m, :nn],
                                    start=(m == 0), stop=(m == 1))
                            for m in range(2):
                                nc.tensor.matmul(
                                    out=ssq_ps[:, :nn], lhsT=ones_col[:],
                                    rhs=sq[:, m, :nn],
                                    start=(m == 0), stop=(m == 1))
                            rows = mt.tile([1, 4, NCHUNK], dt.float32,
                                           tag="rows")
                            rows16 = mt.tile([1, 2, NCHUNK], dt.float16,
                                             tag="rows16")
                            mrow = rows[:, 0, :nn]
                            vrow = rows[:, 1, :nn]
                            rstd = rows[:, 2, :nn]
                            shift = rows[:, 3, :nn]
                            nc.vector.tensor_scalar(
                                out=mrow, in0=mean_ps[:, :nn], scalar1=1.0 / H,
                                scalar2=None, op0=Alu.mult)
                            nc.vector.tensor_tensor(
                                out=rstd, in0=mrow, in1=mrow, op=Alu.mult)
                            nc.vector.scalar_tensor_tensor(
                                out=vrow, in0=ssq_ps[:, :nn], scalar=1.0 / H,
                                in1=rstd, op0=Alu.mult, op1=Alu.subtract)
                            nc.scalar.activation(vrow, vrow, Act.Sqrt,
                                                 bias=eps_sc[:])
                            nc.vector.reciprocal(rstd, vrow)
                            nc.vector.tensor_tensor(
                                out=shift, in0=mrow, in1=rstd, op=Alu.mult)
                            nc.vector.tensor_scalar(
                                out=rows16[:, :, :nn], in0=rows[:, 2:4, :nn],
                                scalar1=1.0, scalar2=None, op0=Alu.mult)
                            bc = pk.tile([128, NCHUNK], dt.float32, tag="pk")
                            bc2 = pk.tile([128, NCHUNK], dt.float32, tag="pk")
                            nc.tensor.matmul(out=bc[:, :nn], lhsT=ones_row[:],
                                             rhs=rows16[:, 0, :nn],
                                             start=True, stop=True)
                            nc.tensor.matmul(out=bc2[:, :nn], lhsT=ones_row[:],
                                             rhs=rows16[:, 1, :nn],
                                             start=True, stop=True)
                            for m in range(2):
                                t1 = u_sb[:, m, :nn]
                                nc.vector.tensor_tensor(
                                    out=t1, in0=t1, in1=bc[:, :nn],
                                    op=Alu.mult)
                                nc.vector.tensor_tensor(
                                    out=t1, in0=t1, in1=bc2[:, :nn],
                                    op=Alu.subtract)
                                nc.vector.tensor_scalar(
                                    out=out_sb[:, m, :nn], in0=t1,
                                    scalar1=colk(gname, m),
                                    scalar2=colk(bename, m),
                                    op0=Alu.mult, op1=Alu.add)
                            if silu:
                                nc.scalar.activation(
                                    out_sb[:, :, :nn], out_sb[:, :, :nn],
                                    Act.Silu)

                        # ---- C: grouped gathers + agg, MLP interleaved ----
                        grp_blocks = shared["grp_blocks"]
                        na, nbv = shared["na"], shared["nb"]
                        a_pos, b_pos = shared["a_pos"], shared["b_pos"]
                        gbase = shared["gbase"]
                        gch_max = shared["gch_max"]
                        mlp_next = 0

                        def emit_mlp_upto(covered_cols):
                            nonlocal mlp_next
                            while mlp_next < len(chunks):
                                n0, nn = chunks[mlp_next]
                                if skip_mlp:
                                    mlp_next += 1
                                    continue
                                if n0 + nn > covered_cols:
                                    return
                                mlp_chunk(n0, nn)
                                mlp_next += 1

                        for g, bl in enumerate(
                                () if skip_agg else grp_blocks):
                            gt = gtp.tile([128, gch_max, H], dt.float16,
                                          tag="gt")
                            ca_g = int(na[g]) // 128
                            cb_g = ca_g + int(nbv[g]) // 128
                            if na[g] > 0 and not skip_gather:
                                nc.gpsimd.dma_gather(
                                    gt[:, 0:ca_g, :], table0[:, :],
                                    idx_sb[:, int(idx_off[g, 0]):
                                           int(idx_off[g, 0] + na[g] // 16)],
                                    int(na[g]), int(na[g]), H,
                                    elem_step=H, single_packet=False,
                                    queue_num=(g % 2) * 2)
                            if nbv[g] > 0 and not skip_gather:
                                nc.gpsimd.dma_gather(
                                    gt[:, ca_g:cb_g, :], table1[:, :],
                                    idx_sb[:, int(idx_off[g, 1]):
                                           int(idx_off[g, 1] + nbv[g] // 16)],
                                    int(nbv[g]), int(nbv[g]), H,
                                    elem_step=H, single_packet=False,
                                    queue_num=(g % 2) * 2 + 1)
                            if agg_dma_only:
                                continue
                            for b in bl:
                                r0 = b * 128
                                rn = min(128, S - r0)
                                poss = (list(range(int(a_pos[b]),
                                                   int(a_pos[b] + NBH[b, 0] // 128)))
                                        + list(range(int(b_pos[b]),
                                                     int(b_pos[b] + NBH[b, 1] // 128))))
                                st = stp.tile([128, ch_max, 128], dt.float16,
                                              tag="st")
                                if st_unmerged:
                                    for i, pos in enumerate(poss):
                                        nc.vector.tensor_tensor(
                                            out=st[:, i, :],
                                            in0=dloc_sb[:, int(gbase[g]) + pos:
                                                        int(gbase[g]) + pos + 1]
                                            .to_broadcast([128, 128]),
                                            in1=iota_sb[:],
                                            op=Alu.is_equal)
                                else:
                                    ca_b = int(NBH[b, 0]) // 128
                                    cb_b = int(NBH[b, 1]) // 128
                                    secs = []
                                    if ca_b > 0:
                                        secs.append((0, int(a_pos[b]), ca_b))
                                    if cb_b > 0:
                                        secs.append((ca_b, int(b_pos[b]), cb_b))
                                    for s_off, pos0, nch in secs:
                                        dl = dloc_sb[:, int(gbase[g]) + pos0:
                                                     int(gbase[g]) + pos0 + nch]
                                        nc.vector.tensor_tensor(
                                            out=st[:, s_off:s_off + nch, :],
                                            in0=dl.rearrange(
                                                "p (c one) -> p c one", one=1)
                                            .to_broadcast([128, nch, 128]),
                                            in1=iota_sb[:].rearrange(
                                                "p (one j) -> p one j", one=1)
                                            .to_broadcast([128, nch, 128]),
                                            op=Alu.is_equal)
                                agg_ps = pa.tile([128, H], dt.float32,
                                                 tag="agg")
                                for i, pos in enumerate(poss):
                                    nc.tensor.matmul(
                                        out=agg_ps[:],
                                        lhsT=st[:, i, :],
                                        rhs=gt[:, pos, :],
                                        start=(i == 0),
                                        stop=(i == len(poss) - 1))
                                aggs = xwp.tile([128, H],
                                                dt.float32 if tp32
                                                else dt.float16, tag="aggs")
                                nc.vector.tensor_scalar(
                                    out=aggs[:], in0=agg_ps[:],
                                    scalar1=dinv_sb[:, b:b + 1], scalar2=None,
                                    op0=Alu.mult)
                                for k in range(2):
                                    tp = pa.tile([128, 128],
                                                 dt.float32 if tp32
                                                 else dt.float16,
                                                 tag="tp", bufs=2)
                                    nc.tensor.transpose(
                                        out=tp[:],
                                        in_=aggs[:, k * 128:(k + 1) * 128],
                                        identity=ident[:])
                                    nc.vector.scalar_tensor_tensor(
                                        out=h_fm[:, k, r0:r0 + rn],
                                        in0=tp[:, :rn],
                                        scalar=colk(f"gcn_b{l}", k),
                                        in1=h_fm[:, k, r0:r0 + rn],
                                        op0=Alu.add, op1=Alu.add)
                            emit_mlp_upto((bl[-1] + 1) * 128)
                        emit_mlp_upto(S)

                    # ================= output projection =================
                    for n0, nn in chunks:
                        uo = mt.tile([128, 2, NCHUNK], dt.float16, tag="u")
                        for m in range(2):
                            ps = pk.tile([128, NCHUNK], dt.float32, tag="pk")
                            for k in range(2):
                                nc.tensor.matmul(
                                    out=ps[:, :nn],
                                    lhsT=out_w1_sb[:, k, m * 128:(m + 1) * 128],
                                    rhs=h_fm[:, k, n0:n0 + nn],
                                    start=(k == 0), stop=(k == 1))
                            nc.scalar.activation(
                                uo[:, m, :nn], ps[:, :nn], Act.Silu,
                                bias=colk("out_b1", m))
                        yps = pk.tile([128, NCHUNK], dt.float32, tag="pk")
                        for k in range(2):
                            nc.tensor.matmul(
                                out=yps[:, :nn],
                                lhsT=out_w2_sb[:, k, :],
                                rhs=uo[:, k, :nn],
                                start=(k == 0), stop=(k == 1))
                        ysb = mt.tile([128, NCHUNK], dt.float32, tag="ysb", bufs=1)
                        nc.vector.tensor_scalar_add(
                            ysb[:, :nn], yps[:, :nn], col("out_b2"))
                        nc.sync.dma_start(out=y_out[:, n0:n0 + nn],
                                          in_=ysb[:, :nn])
    nc.compile()
    return nc


def make_in_maps(inputs, shared, per_core, cmap_cols, rmap_rows=None):
    cv, _ = cmap_cols
    x = np.asarray(inputs["x"], np.float32)
    common = dict(
        cv=cv,
        in_w=np.asarray(inputs["in_w"], _FP16),
        gcn_w=np.asarray(inputs["gcn_w"], _FP16),
        res_w1=np.asarray(inputs["res_w1"], _FP16),
        res_w2=np.asarray(inputs["res_w2"], _FP16),
        out_w1=np.asarray(inputs["out_w1"], _FP16),
        out_w2=np.asarray(inputs["out_w2"], _FP16),
    )
    in_maps = []
    for c in range(C):
        pc = per_core[c]
        m = dict(common)
        m["xT"] = np.ascontiguousarray(x[c * S:(c + 1) * S, :].T).astype(_FP16)
        m["idx"] = pc["idx"]
        m["dloc"] = pc["dloc"]
        m["dinv"] = pc["dinv"]
        m["iota"] = np.tile(np.arange(128, dtype=np.float16)[None, :], (128, 1))
        in_maps.append(m)
    return in_maps


_BUILD_CACHE = {}


def _get_module(shared, cmap, rmap, reps):
    key = (reps, shared["icols"], shared["nch_tot"])
    if key not in _BUILD_CACHE:
        _BUILD_CACHE[key] = build_module(shared, cmap, rmap, reps)
    return _BUILD_CACHE[key]


def build_rowvecs(inputs):
    # kept for harness compatibility; v4 uses column vectors only
    return np.zeros((1, 1), _FP16), {}


def kernel(**inputs):
    from concourse.bass_utils import run_bass_kernel_spmd

    shared, per_core = preprocess(np.asarray(inputs["edge_index"]))
    cv, cmap = build_colvecs(inputs)
    cmap = dict(cmap)
    cmap["ncols"] = cv.shape[1]
    nc = _get_module(shared, cmap, None, reps=1)
    in_maps = make_in_maps(inputs, shared, per_core, (cv, cmap))
    res = run_bass_kernel_spmd(nc, in_maps, core_ids=list(range(C)))
    y = np.concatenate([r["y"].T for r in res.results], axis=0)
    return y.astype(np.float32)

